# revision 20
# baseline (speedup 1.0000x reference)
"""DFMConv2d Trainium2 kernel (v2: dy-accumulated conv, bf16 I/O).

Reference computation (per sample b):
  pooled = mean_{h,w} x[b]                          [C=256]
  h      = relu(pooled @ w1.T + b1)                 [128]
  mix    = softmax((h @ w2.T + b2).reshape(256, 8)) [256, 8]
  y      = conv3x3_SAME(x[b], base_filters)         [8, 64, 64]
  out[b] = einsum('on,nhw->ohw', mix, y)            [256, 64, 64]

Strategy (8 cores, data-parallel over batch, 8 samples/core), bf16:

  conv:  stationary M=24 holds (dx, n); the three dy taps ACCUMULATE into
         one PSUM [24, 512] via +/-64-column offsets on the moving x
         operand (row shifts are free; hc-edge blocks use partial column
         ranges so x needs no padding and stays 16KB-contiguous).
  shift: only dx = +/-1 column remains: 3 engine copies (DVE + GpSimd)
         z24 -> zs, plus tiny wrap-column memsets. No SBUF->SBUF DMAs.
  mix:   out[o, hw] = mixT24.T @ zs with K=24; output channel o = 2p+oc
         folded into the w2 column permutation so stores are contiguous.
  MLP:   batched 4 samples at a time; softmax normalization deferred to
         the final PSUM->SBUF copy via per-partition scale (1/sum).
  DMA:   x loads on GpSimd SWDGE ring (sample 0 on sync HWDGE), out
         stores on scalar HWDGE ring; everything bf16 => 2MB/sample each
         way in 16KB-contiguous runs.
"""
import sys

sys.path.insert(0, "/opt/trn_rl_repo")

import numpy as np
import ml_dtypes

import concourse.bass as bass
import concourse.bacc as bacc
import concourse.tile as tile
import concourse.mybir as mybir
from concourse.bass_utils import run_bass_kernel_spmd
from contextlib import ExitStack

F32 = mybir.dt.float32
BF16 = mybir.dt.bfloat16
AFT = mybir.ActivationFunctionType
AXX = mybir.AxisListType.X
ALU = mybir.AluOpType

N_CORES = 8
BPC = 8            # samples per core
C = 256
CO = 256
H = W = 64
HW = H * W
NB = 8             # n_base
HID = 128
M72 = 72           # stationary rows: 32*dx + n (32-aligned dx groups)
ZLEN = 1 + HW + 1  # z24 row: lead zero col + 4096 + tail zero col

_BUILT = None


def _build():
    nc = bacc.Bacc("TRN2", target_bir_lowering=False)

    d_x = nc.dram_tensor("x", [128, BPC, 2, HW], BF16, kind="ExternalInput")
    d_ft = nc.dram_tensor("ft", [128, 2, 3, 128], BF16, kind="ExternalInput")
    d_w1tb = nc.dram_tensor("w1tb", [128, 2, HID], BF16, kind="ExternalInput")
    d_b1 = nc.dram_tensor("b1", [HID, 1], F32, kind="ExternalInput")
    d_w2p = nc.dram_tensor("w2p", [HID, 2, NB, 128], F32, kind="ExternalInput")
    d_b2r = nc.dram_tensor("b2r", [128, 2, NB, BPC], F32, kind="ExternalInput")
    d_id = nc.dram_tensor("ident", [128, 128], F32, kind="ExternalInput")
    d_out = nc.dram_tensor("out", [128, BPC, 2, HW], BF16, kind="ExternalOutput")

    with tile.TileContext(nc) as tc, ExitStack() as ctx:
        prm = ctx.enter_context(tc.tile_pool(name="prm", bufs=1))
        xp = ctx.enter_context(tc.tile_pool(name="xp", bufs=3))
        z24p = ctx.enter_context(tc.tile_pool(name="z24p", bufs=2))
        zsp = ctx.enter_context(tc.tile_pool(name="zsp", bufs=6))
        op = ctx.enter_context(tc.tile_pool(name="op", bufs=3))
        mtp = ctx.enter_context(tc.tile_pool(name="mtp", bufs=2))
        sm = ctx.enter_context(tc.tile_pool(name="sm", bufs=2))
        ps_c = ctx.enter_context(tc.tile_pool(name="ps_c", bufs=3, space="PSUM"))
        ps_m = ctx.enter_context(tc.tile_pool(name="ps_m", bufs=2, space="PSUM"))
        ps_s = ctx.enter_context(tc.tile_pool(name="ps_s", bufs=1, space="PSUM"))
        ps_u = ctx.enter_context(tc.tile_pool(name="ps_u", bufs=1, space="PSUM"))

        # ---- params (loaded once, sync HW ring) ----
        ft_sb = prm.tile([128, 2, 3, 128], BF16, tag="ft")
        nc.sync.dma_start(out=ft_sb, in_=d_ft[:, :, :, :])
        w1tb_sb = prm.tile([128, 2, HID], BF16, tag="w1tb")
        nc.sync.dma_start(out=w1tb_sb, in_=d_w1tb[:, :, :])
        b1_sb = prm.tile([HID, 1], F32, tag="b1")
        nc.sync.dma_start(out=b1_sb, in_=d_b1[:, :])
        w2p_sb = prm.tile([HID, 2, NB, 128], F32, tag="w2p")
        nc.sync.dma_start(out=w2p_sb, in_=d_w2p[:, :, :, :])
        b2r_sb = prm.tile([128, 2, NB, BPC], F32, tag="b2r")
        nc.sync.dma_start(out=b2r_sb, in_=d_b2r[:, :, :, :])
        id_sb = prm.tile([128, 128], F32, tag="ident")
        nc.sync.dma_start(out=id_sb, in_=d_id[:, :])

        hraw_sb = prm.tile([HID, BPC], F32, tag="hraw")
        mr_sb = prm.tile([128, M72], F32, tag="mr")
        nc.vector.memset(mr_sb, 0)   # gap columns stay zero forever
        h_sb = prm.tile([HID, BPC], F32, tag="h")

        # HAM warm-up: ~20 junk matmuls keep the PE busy during the first
        # x load so real work starts at 2.4 GHz
        for w in range(36):
            psw = ps_s.tile([128, 512], F32, tag="s")
            nc.tensor.matmul(psw[:, 0:384], ft_sb[:, 0, 0, :],
                             ft_sb[:, 1, :, :].rearrange("p a b -> p (a b)"),
                             start=True, stop=True)

        xts = [None] * BPC
        zss = [None] * BPC
        GROUPS = [(0, 4), (4, 2), (6, 2)]   # (j0, nj) MLP groups
        mixTs = [None] * 3
        recs = [None] * 3

        def mlp_batch(b):
            j0, nj = GROUPS[b]
            nc.scalar.activation(out=h_sb[:, j0:j0 + nj],
                                 in_=hraw_sb[:, j0:j0 + nj], func=AFT.Relu,
                                 bias=b1_sb, scale=1.0)
            ps = ps_s.tile([128, 512], F32, tag="s")
            pl = ps[:, 0:16 * nj].rearrange("p (a n j) -> p a n j", a=2, n=NB)
            for oc in range(2):
                for n in range(NB):
                    nc.tensor.matmul(pl[:, oc, n, :], w2p_sb[:, oc, n, :],
                                     h_sb[:, j0:j0 + nj], start=True, stop=True)
            lg = sm.tile([128, 2, NB, nj], F32, tag="lg")
            nc.vector.tensor_tensor(out=lg, in0=pl, in1=b2r_sb[:, :, :, 0:nj],
                                    op=ALU.add)
            ex = sm.tile([128, 2, NB, nj], F32, tag="ex")
            nc.scalar.activation(out=ex, in_=lg, func=AFT.Exp)
            # transpose (n, j) -> (j, n) so the n-sum is innermost
            exT = sm.tile([128, 2, nj, NB], F32, tag="exT")
            nc.vector.tensor_copy(exT, ex.rearrange("p a n j -> p a j n"))
            sums = sm.tile([128, 2, nj], F32, tag="sums")
            nc.vector.reduce_sum(sums, exT, axis=AXX)
            rec_b = mtp.tile([128, 2, nj], F32, tag="rec")
            nc.vector.reciprocal(rec_b, sums)
            recs[b] = rec_b
            mixT_b = mtp.tile([M72, 2, nj, 128], BF16, tag="mixT")
            mixTs[b] = mixT_b
            for j2 in range(nj):
                for oc in range(2):
                    for dx in range(3):
                        nc.vector.tensor_copy(mr_sb[:, 32 * dx:32 * dx + 8],
                                              exT[:, oc, j2, :])
                    ps2 = ps_s.tile([128, 512], F32, tag="s")
                    ptr = ps2[0:M72, 128:256]
                    nc.tensor.transpose(ptr, mr_sb, id_sb)
                    nc.vector.tensor_copy(mixT_b[:, oc, j2, :], ptr)

        def emit_mix(b):
            j0, nj = GROUPS[b]
            mixT_b = mixTs[b]
            rec_b = recs[b]
            for j2 in range(nj):
                jj = j0 + j2
                zsj = zss[jj]
                ot = op.tile([128, 2, HW], BF16, tag="out")
                for oc in range(2):
                    for hc in range(8):
                        om = ps_m.tile([128, 512], F32, tag="om")
                        nc.tensor.matmul(om, mixT_b[:, oc, j2, :],
                                         zsj[:, 512 * hc:512 * (hc + 1)],
                                         start=True, stop=True)
                        dst = ot[:, oc, 512 * hc:512 * (hc + 1)]
                        if hc % 2 == 0:
                            nc.vector.tensor_scalar_mul(
                                out=dst, in0=om,
                                scalar1=rec_b[:, oc, j2:j2 + 1])
                        else:
                            nc.scalar.activation(
                                out=dst, in_=om, func=AFT.Copy,
                                scale=rec_b[:, oc, j2:j2 + 1])
                oring = nc.gpsimd if jj == 0 else nc.scalar
                oring.dma_start(out=d_out[:, jj, :, :], in_=ot)

        for j in range(BPC):
            # ---- load x (SWDGE ring; first sample on sync HW ring) ----
            xt = xp.tile([128, 2, HW], BF16, tag="x")
            xts[j] = xt
            nc.gpsimd.dma_start(out=xt, in_=d_x[:, j, :, :])

            # ---- conv: accumulate 3 dy taps x 2 channel halves in PSUM ----
            z24 = z24p.tile([M72, ZLEN], BF16, tag="z24")
            if j < 2:
                nc.vector.memset(z24[:, 0:1], 0)
                nc.vector.memset(z24[:, ZLEN - 1:ZLEN], 0)
            ups = ps_u.tile([HID, 512], F32, tag="u")
            for hc in range(8):
                yps = ps_c.tile([128, 512], F32, tag="yps")
                dys = (1, 2, 0) if hc == 7 else (1, 0, 2)
                k = 0
                for dy in dys:
                    lo = 512 * hc + 64 * (dy - 1)
                    hi = lo + 512
                    clo, chi = max(lo, 0), min(hi, HW)
                    for half in range(2):
                        nc.tensor.matmul(
                            yps[:, clo - lo:512 - (hi - chi)],
                            ft_sb[:, half, dy, :],
                            xt[:, half, clo:chi],
                            start=(k == 0), stop=(k == 5))
                        k += 1
                for half in range(2):
                    nc.tensor.matmul(
                        ups, w1tb_sb[:, half, :],
                        xt[:, half, 512 * hc:512 * (hc + 1)],
                        start=(hc == 0 and half == 0),
                        stop=(hc == 7 and half == 1))
                nc.scalar.copy(
                    out=z24[:, 1 + 512 * hc:1 + 512 * (hc + 1)], in_=yps[0:72, :])
            nc.vector.reduce_sum(hraw_sb[:, j:j + 1], ups, axis=AXX)

            # ---- dx shift: z24 -> zs on DVE/GpSimd, then wrap fixups ----
            zs = zsp.tile([M72, HW], BF16, tag="zs")
            zss[j] = zs
            # first use of each pool buffer: copy the (zero) gap rows too so
            # every zs row is initialized; afterwards gaps stay zero
            w0 = 32 if j < 6 else 8
            w1 = 32 if j < 6 else 8
            nc.sync.dma_start(out=zs[0:w0, :], in_=z24[0:w0, 0:HW])
            nc.sync.dma_start(out=zs[32:32 + w1, :], in_=z24[32:32 + w1, 1:1 + HW])
            nc.sync.dma_start(out=zs[64:72, :], in_=z24[64:72, 2:2 + HW])
            zv = zs.rearrange("p (h w) -> p h w", w=64)
            nc.vector.memset(zv[0:8, :, 0:1], 0)
            nc.vector.memset(zv[64:72, :, 63:64], 0)

            # ---- mix deferred so conv matmuls fill the MLP latency ----
            if j == 4:
                mlp_batch(0)
            if j == 5:
                emit_mix(0)
            if j == 6:
                mlp_batch(1)
            if j == 7:
                emit_mix(1)
                mlp_batch(2)
                emit_mix(2)

    nc.compile()
    return nc


def _prep_inputs(x, w1, b1, w2, b2, base_filters):
    """Host-side input layout prep. Returns per-core in_maps."""
    B = x.shape[0]
    # x[core*8+j, 2p+half, hw] -> [128(p), 8(j), 2(half), hw] bf16
    xs = x.reshape(B, 128, 2, HW)
    # w1t[p, half, hid] = w1[hid, 2p+half] / HW  (mean folded in)
    w1t = (np.ascontiguousarray(
        w1.reshape(HID, 128, 2).transpose(1, 2, 0)).astype(np.float32)
        / float(HW)).astype(ml_dtypes.bfloat16)
    b1c = np.ascontiguousarray(b1.reshape(HID, 1)).astype(np.float32)
    # w2 row index = o*NB + n with o = 2p+oc -> w2p[hid, oc, n, p]
    w2r = w2.reshape(128, 2, NB, HID)          # [p, oc, n, hid]
    w2p = np.ascontiguousarray(w2r.transpose(3, 1, 2, 0)).astype(np.float32)
    b2r = np.broadcast_to(
        b2.reshape(128, 2, NB, 1), (128, 2, NB, BPC))
    b2r = np.ascontiguousarray(b2r).astype(np.float32)
    # ft[p, half, dy, 32dx+n] = filt[n, 2p+half, dy, dx]; gaps zero
    filt = base_filters.reshape(NB, 128, 2, 3, 3)
    fpv = filt.transpose(1, 2, 3, 4, 0)                 # [p, half, dy, dx, n]
    ft = np.zeros((128, 2, 3, 128), dtype=np.float32)
    for dx in range(3):
        ft[:, :, :, 32 * dx:32 * dx + NB] = fpv[:, :, :, dx, :]
    ft = ft.astype(ml_dtypes.bfloat16)
    ident = np.eye(128, dtype=np.float32)

    in_maps = []
    for core in range(N_CORES):
        xc = np.ascontiguousarray(
            xs[core * BPC:(core + 1) * BPC].transpose(1, 0, 2, 3)).astype(
                ml_dtypes.bfloat16)
        in_maps.append({
            "x": xc, "ft": ft, "w1tb": w1t, "b1": b1c, "w2p": w2p,
            "b2r": b2r, "ident": ident,
        })
    return in_maps


def kernel(x, w1, b1, w2, b2, base_filters):
    global _BUILT
    if _BUILT is None:
        _BUILT = _build()
    nc = _BUILT
    in_maps = _prep_inputs(np.asarray(x, dtype=np.float32),
                           np.asarray(w1, dtype=np.float32),
                           np.asarray(b1, dtype=np.float32),
                           np.asarray(w2, dtype=np.float32),
                           np.asarray(b2, dtype=np.float32),
                           np.asarray(base_filters, dtype=np.float32))
    res = run_bass_kernel_spmd(nc, in_maps, core_ids=list(range(N_CORES)))
    outs = []
    for core in range(N_CORES):
        o = res.results[core]["out"]            # [128, BPC, 2, HW] bf16
        o = np.asarray(o).astype(np.float32).transpose(1, 0, 2, 3)
        outs.append(o.reshape(BPC, CO, H, W))
    return np.concatenate(outs, axis=0).astype(np.float32)


# revision 21
# speedup vs baseline: 1.0398x; 1.0398x over previous
"""DFMConv2d Trainium2 kernel (v2: dy-accumulated conv, bf16 I/O).

Reference computation (per sample b):
  pooled = mean_{h,w} x[b]                          [C=256]
  h      = relu(pooled @ w1.T + b1)                 [128]
  mix    = softmax((h @ w2.T + b2).reshape(256, 8)) [256, 8]
  y      = conv3x3_SAME(x[b], base_filters)         [8, 64, 64]
  out[b] = einsum('on,nhw->ohw', mix, y)            [256, 64, 64]

Strategy (8 cores, data-parallel over batch, 8 samples/core), bf16:

  conv:  stationary M=24 holds (dx, n); the three dy taps ACCUMULATE into
         one PSUM [24, 512] via +/-64-column offsets on the moving x
         operand (row shifts are free; hc-edge blocks use partial column
         ranges so x needs no padding and stays 16KB-contiguous).
  shift: only dx = +/-1 column remains: 3 engine copies (DVE + GpSimd)
         z24 -> zs, plus tiny wrap-column memsets. No SBUF->SBUF DMAs.
  mix:   out[o, hw] = mixT24.T @ zs with K=24; output channel o = 2p+oc
         folded into the w2 column permutation so stores are contiguous.
  MLP:   batched 4 samples at a time; softmax normalization deferred to
         the final PSUM->SBUF copy via per-partition scale (1/sum).
  DMA:   x loads on GpSimd SWDGE ring (sample 0 on sync HWDGE), out
         stores on scalar HWDGE ring; everything bf16 => 2MB/sample each
         way in 16KB-contiguous runs.
"""
import sys

sys.path.insert(0, "/opt/trn_rl_repo")

import numpy as np
import ml_dtypes

import concourse.bass as bass
import concourse.bacc as bacc
import concourse.tile as tile
import concourse.mybir as mybir
from concourse.bass_utils import run_bass_kernel_spmd
from contextlib import ExitStack

F32 = mybir.dt.float32
BF16 = mybir.dt.bfloat16
AFT = mybir.ActivationFunctionType
AXX = mybir.AxisListType.X
ALU = mybir.AluOpType

N_CORES = 8
BPC = 8            # samples per core
C = 256
CO = 256
H = W = 64
HW = H * W
NB = 8             # n_base
HID = 128
M72 = 72           # stationary rows: 32*dx + n (32-aligned dx groups)
ZLEN = 1 + HW + 1  # z24 row: lead zero col + 4096 + tail zero col

_BUILT = None


def _build():
    nc = bacc.Bacc("TRN2", target_bir_lowering=False)

    d_x = nc.dram_tensor("x", [128, BPC, 2, HW], BF16, kind="ExternalInput")
    d_ft = nc.dram_tensor("ft", [128, 2, 3, 128], BF16, kind="ExternalInput")
    d_w1tb = nc.dram_tensor("w1tb", [128, 2, HID], BF16, kind="ExternalInput")
    d_b1 = nc.dram_tensor("b1", [HID, 1], F32, kind="ExternalInput")
    d_w2p = nc.dram_tensor("w2p", [HID, 2, NB, 128], F32, kind="ExternalInput")
    d_b2r = nc.dram_tensor("b2r", [128, 2, NB, BPC], F32, kind="ExternalInput")
    d_id = nc.dram_tensor("ident", [128, 128], F32, kind="ExternalInput")
    d_out = nc.dram_tensor("out", [128, BPC, 2, HW], BF16, kind="ExternalOutput")

    with tile.TileContext(nc) as tc, ExitStack() as ctx:
        prm = ctx.enter_context(tc.tile_pool(name="prm", bufs=1))
        xp = ctx.enter_context(tc.tile_pool(name="xp", bufs=3))
        z24p = ctx.enter_context(tc.tile_pool(name="z24p", bufs=2))
        zsp = ctx.enter_context(tc.tile_pool(name="zsp", bufs=6))
        op = ctx.enter_context(tc.tile_pool(name="op", bufs=3))
        mtp = ctx.enter_context(tc.tile_pool(name="mtp", bufs=2))
        sm = ctx.enter_context(tc.tile_pool(name="sm", bufs=2))
        ps_c = ctx.enter_context(tc.tile_pool(name="ps_c", bufs=3, space="PSUM"))
        ps_m = ctx.enter_context(tc.tile_pool(name="ps_m", bufs=3, space="PSUM"))
        ps_s = ctx.enter_context(tc.tile_pool(name="ps_s", bufs=1, space="PSUM"))
        ps_u = ctx.enter_context(tc.tile_pool(name="ps_u", bufs=1, space="PSUM"))

        # ---- params (loaded once, sync HW ring) ----
        ft_sb = prm.tile([128, 2, 3, 128], BF16, tag="ft")
        nc.sync.dma_start(out=ft_sb, in_=d_ft[:, :, :, :])
        w1tb_sb = prm.tile([128, 2, HID], BF16, tag="w1tb")
        nc.sync.dma_start(out=w1tb_sb, in_=d_w1tb[:, :, :])
        b1_sb = prm.tile([HID, 1], F32, tag="b1")
        nc.sync.dma_start(out=b1_sb, in_=d_b1[:, :])
        w2p_sb = prm.tile([HID, 2, NB, 128], F32, tag="w2p")
        nc.sync.dma_start(out=w2p_sb, in_=d_w2p[:, :, :, :])
        b2r_sb = prm.tile([128, 2, NB, BPC], F32, tag="b2r")
        nc.sync.dma_start(out=b2r_sb, in_=d_b2r[:, :, :, :])
        id_sb = prm.tile([128, 128], F32, tag="ident")
        nc.sync.dma_start(out=id_sb, in_=d_id[:, :])

        hraw_sb = prm.tile([HID, BPC], F32, tag="hraw")
        mr_sb = prm.tile([128, M72], F32, tag="mr")
        nc.vector.memset(mr_sb, 0)   # gap columns stay zero forever
        h_sb = prm.tile([HID, BPC], F32, tag="h")

        # HAM warm-up: junk matmuls keep the PE busy during the first
        # x load so real work starts at 2.4 GHz; cycle the pool so they
        # pipeline instead of WAR-serializing
        ftf = ft_sb.rearrange("p a b c -> p (a b c)")
        for w in range(20):
            psw = ps_c.tile([128, 512], F32, tag="yps")
            nc.tensor.matmul(psw, ft_sb[:, 0, 0, :], ftf[:, 0:512],
                             start=True, stop=True)

        xts = [None] * BPC
        zss = [None] * BPC
        GROUPS = [(0, 2), (2, 2), (4, 2), (6, 2)]   # (j0, nj) MLP groups
        mixTs = [None] * 4
        recs = [None] * 4

        def mlp_batch(b):
            j0, nj = GROUPS[b]
            nc.scalar.activation(out=h_sb[:, j0:j0 + nj],
                                 in_=hraw_sb[:, j0:j0 + nj], func=AFT.Relu,
                                 bias=b1_sb, scale=1.0)
            ps = ps_s.tile([128, 512], F32, tag="s")
            pl = ps[:, 0:16 * nj].rearrange("p (a n j) -> p a n j", a=2, n=NB)
            for oc in range(2):
                for n in range(NB):
                    nc.tensor.matmul(pl[:, oc, n, :], w2p_sb[:, oc, n, :],
                                     h_sb[:, j0:j0 + nj], start=True, stop=True)
            lg = sm.tile([128, 2, NB, nj], F32, tag="lg")
            nc.vector.tensor_tensor(out=lg, in0=pl, in1=b2r_sb[:, :, :, 0:nj],
                                    op=ALU.add)
            ex = sm.tile([128, 2, NB, nj], F32, tag="ex")
            nc.scalar.activation(out=ex, in_=lg, func=AFT.Exp)
            # transpose (n, j) -> (j, n) so the n-sum is innermost
            exT = sm.tile([128, 2, nj, NB], F32, tag="exT")
            nc.vector.tensor_copy(exT, ex.rearrange("p a n j -> p a j n"))
            sums = sm.tile([128, 2, nj], F32, tag="sums")
            nc.vector.reduce_sum(sums, exT, axis=AXX)
            rec_b = mtp.tile([128, 2, nj], F32, tag="rec")
            nc.vector.reciprocal(rec_b, sums)
            recs[b] = rec_b
            mixT_b = mtp.tile([M72, 2, nj, 128], BF16, tag="mixT")
            mixTs[b] = mixT_b
            for j2 in range(nj):
                for oc in range(2):
                    for dx in range(3):
                        nc.vector.tensor_copy(mr_sb[:, 32 * dx:32 * dx + 8],
                                              exT[:, oc, j2, :])
                    ps2 = ps_s.tile([128, 512], F32, tag="s")
                    ptr = ps2[0:M72, 128:256]
                    nc.tensor.transpose(ptr, mr_sb, id_sb)
                    nc.vector.tensor_copy(mixT_b[:, oc, j2, :], ptr)

        def emit_mix(b):
            j0, nj = GROUPS[b]
            mixT_b = mixTs[b]
            rec_b = recs[b]
            for j2 in range(nj):
                jj = j0 + j2
                zsj = zss[jj]
                ot = op.tile([128, 2, HW], BF16, tag="out")
                for oc in range(2):
                    for hc in range(8):
                        om = ps_m.tile([128, 512], F32, tag="om")
                        nc.tensor.matmul(om, mixT_b[:, oc, j2, :],
                                         zsj[:, 512 * hc:512 * (hc + 1)],
                                         start=True, stop=True)
                        dst = ot[:, oc, 512 * hc:512 * (hc + 1)]
                        if hc % 2 == 0:
                            nc.vector.tensor_scalar_mul(
                                out=dst, in0=om,
                                scalar1=rec_b[:, oc, j2:j2 + 1])
                        else:
                            nc.scalar.activation(
                                out=dst, in_=om, func=AFT.Copy,
                                scale=rec_b[:, oc, j2:j2 + 1])
                oring = nc.gpsimd if jj == 7 else nc.scalar
                oring.dma_start(out=d_out[:, jj, :, :], in_=ot)

        for j in range(BPC):
            # ---- load x (SWDGE ring; first sample on sync HW ring) ----
            xt = xp.tile([128, 2, HW], BF16, tag="x")
            xts[j] = xt
            if j == 0:
                nc.sync.dma_start(out=xt[:, 0, :], in_=d_x[:, j, 0, :])
                nc.gpsimd.dma_start(out=xt[:, 1, :], in_=d_x[:, j, 1, :])
            elif j == 1:
                nc.sync.dma_start(out=xt, in_=d_x[:, j, :, :])
            else:
                nc.gpsimd.dma_start(out=xt, in_=d_x[:, j, :, :])

            # ---- conv: accumulate 3 dy taps x 2 channel halves in PSUM ----
            z24 = z24p.tile([M72, ZLEN], BF16, tag="z24")
            if j < 2:
                nc.vector.memset(z24[:, 0:1], 0)
                nc.vector.memset(z24[:, ZLEN - 1:ZLEN], 0)
            ups = ps_u.tile([HID, 512], F32, tag="u")
            for hc in range(8):
                yps = ps_c.tile([128, 512], F32, tag="yps")
                dys = (1, 2, 0) if hc == 7 else (1, 0, 2)
                k = 0
                for dy in dys:
                    lo = 512 * hc + 64 * (dy - 1)
                    hi = lo + 512
                    clo, chi = max(lo, 0), min(hi, HW)
                    for half in range(2):
                        nc.tensor.matmul(
                            yps[:, clo - lo:512 - (hi - chi)],
                            ft_sb[:, half, dy, :],
                            xt[:, half, clo:chi],
                            start=(k == 0), stop=(k == 5))
                        k += 1
                for half in range(2):
                    nc.tensor.matmul(
                        ups, w1tb_sb[:, half, :],
                        xt[:, half, 512 * hc:512 * (hc + 1)],
                        start=(hc == 0 and half == 0),
                        stop=(hc == 7 and half == 1))
                nc.scalar.copy(
                    out=z24[:, 1 + 512 * hc:1 + 512 * (hc + 1)], in_=yps[0:72, :])
            nc.vector.reduce_sum(hraw_sb[:, j:j + 1], ups, axis=AXX)

            # ---- dx shift: z24 -> zs on DVE/GpSimd, then wrap fixups ----
            zs = zsp.tile([M72, HW], BF16, tag="zs")
            zss[j] = zs
            # first use of each pool buffer: copy the (zero) gap rows too so
            # every zs row is initialized; afterwards gaps stay zero
            w0 = 32 if j < 6 else 8
            w1 = 32 if j < 6 else 8
            nc.sync.dma_start(out=zs[0:w0, :], in_=z24[0:w0, 0:HW])
            nc.sync.dma_start(out=zs[32:32 + w1, :], in_=z24[32:32 + w1, 1:1 + HW])
            nc.sync.dma_start(out=zs[64:72, :], in_=z24[64:72, 2:2 + HW])
            zv = zs.rearrange("p (h w) -> p h w", w=64)
            nc.vector.memset(zv[0:8, :, 0:1], 0)
            nc.vector.memset(zv[64:72, :, 63:64], 0)

            # ---- 2-sample MLP groups; mix + stores spread through kernel ----
            if j in (2, 4, 6):
                g = j // 2 - 1
                mlp_batch(g)
                emit_mix(g)
            if j == 7:
                mlp_batch(3)
                emit_mix(3)

    nc.compile()
    return nc


def _prep_inputs(x, w1, b1, w2, b2, base_filters):
    """Host-side input layout prep. Returns per-core in_maps."""
    B = x.shape[0]
    # x[core*8+j, 2p+half, hw] -> [128(p), 8(j), 2(half), hw] bf16
    xs = x.reshape(B, 128, 2, HW)
    # w1t[p, half, hid] = w1[hid, 2p+half] / HW  (mean folded in)
    w1t = (np.ascontiguousarray(
        w1.reshape(HID, 128, 2).transpose(1, 2, 0)).astype(np.float32)
        / float(HW)).astype(ml_dtypes.bfloat16)
    b1c = np.ascontiguousarray(b1.reshape(HID, 1)).astype(np.float32)
    # w2 row index = o*NB + n with o = 2p+oc -> w2p[hid, oc, n, p]
    w2r = w2.reshape(128, 2, NB, HID)          # [p, oc, n, hid]
    w2p = np.ascontiguousarray(w2r.transpose(3, 1, 2, 0)).astype(np.float32)
    b2r = np.broadcast_to(
        b2.reshape(128, 2, NB, 1), (128, 2, NB, BPC))
    b2r = np.ascontiguousarray(b2r).astype(np.float32)
    # ft[p, half, dy, 32dx+n] = filt[n, 2p+half, dy, dx]; gaps zero
    filt = base_filters.reshape(NB, 128, 2, 3, 3)
    fpv = filt.transpose(1, 2, 3, 4, 0)                 # [p, half, dy, dx, n]
    ft = np.zeros((128, 2, 3, 128), dtype=np.float32)
    for dx in range(3):
        ft[:, :, :, 32 * dx:32 * dx + NB] = fpv[:, :, :, dx, :]
    ft = ft.astype(ml_dtypes.bfloat16)
    ident = np.eye(128, dtype=np.float32)

    in_maps = []
    for core in range(N_CORES):
        xc = np.ascontiguousarray(
            xs[core * BPC:(core + 1) * BPC].transpose(1, 0, 2, 3)).astype(
                ml_dtypes.bfloat16)
        in_maps.append({
            "x": xc, "ft": ft, "w1tb": w1t, "b1": b1c, "w2p": w2p,
            "b2r": b2r, "ident": ident,
        })
    return in_maps


def kernel(x, w1, b1, w2, b2, base_filters):
    global _BUILT
    if _BUILT is None:
        _BUILT = _build()
    nc = _BUILT
    in_maps = _prep_inputs(np.asarray(x, dtype=np.float32),
                           np.asarray(w1, dtype=np.float32),
                           np.asarray(b1, dtype=np.float32),
                           np.asarray(w2, dtype=np.float32),
                           np.asarray(b2, dtype=np.float32),
                           np.asarray(base_filters, dtype=np.float32))
    res = run_bass_kernel_spmd(nc, in_maps, core_ids=list(range(N_CORES)))
    outs = []
    for core in range(N_CORES):
        o = res.results[core]["out"]            # [128, BPC, 2, HW] bf16
        o = np.asarray(o).astype(np.float32).transpose(1, 0, 2, 3)
        outs.append(o.reshape(BPC, CO, H, W))
    return np.concatenate(outs, axis=0).astype(np.float32)
